# revision 3
# baseline (speedup 1.0000x reference)
"""CRF Viterbi decode — v2: Act-producer + DVE stt/reduce hybrid forward.

Forward per step (partitions p=(ic*32+b); group ic reduces i in [64ic,64ic+64)):
  - DVE stt chain for k in [0, N_DVE) -> acc [128,256]
  - Act produces planes k in [N_DVE, 64): ch[:, :, kk] = trans_rep[:,k,:] + s[:,k]
    (strided writes into chunk tiles [128, 256, nk])
  - DVE tensor_reduce max per chunk (contiguous-k inner axis); partials merged
    via TT max (PARTIAL_PLANE=False) or by riding as plane 0 of next chunk
  - TT max with acc -> macc; 4-group combine: Act shift-copies groups 1..3 to
    partition-base 0 tiles, then 3 aligned TT maxes -> m32 [32,256]
  - fold: Act copies m32 col-blocks to partition offsets, DVE adds em -> s_next
  - lat store via DMA (off critical path)
Backtrack: recompute-argmax chains as v1, but argmax via max8/max_index
(first-index ties verified on HW) and idx bitcast uint32->int32 feeds the
SWDGE gather directly.
"""

import numpy as np

B, T, K = 256, 512, 256
NCORES = 8
BLOC = B // NCORES  # 32
OUT_T = T + 2  # 514
BIGN = float(T)  # iota shift: iota_neg = i - 512 (negative for all i < 512)
NCHAIN = 2

N_DVE = 16          # k's on DVE stt chain
CHUNK = 8           # planes per Act chunk (PSUM tile = CHUNK KB/partition)
CH_SPACE = "PSUM"   # chunk tile space: Act strided->PSUM measured 477ns/plane


def build_program(t_steps: int = T):
    from contextlib import ExitStack

    import concourse.bass as bass
    import concourse.tile as tile
    from concourse import bacc, mybir

    FP32 = mybir.dt.float32
    INT32 = mybir.dt.int32
    UINT32 = mybir.dt.uint32
    A = mybir.AluOpType

    n_act = 64 - N_DVE
    chunk_sizes = []
    rem = n_act
    while rem > 0:
        chunk_sizes.append(min(CHUNK, rem))
        rem -= min(CHUNK, rem)

    nc = bacc.Bacc("TRN2", target_bir_lowering=False, num_devices=NCORES)

    em_f_d = nc.dram_tensor("em_f", [t_steps, 128, 64], FP32, kind="ExternalInput")
    trans_rep_d = nc.dram_tensor("trans_rep", [64, 128, K], FP32, kind="ExternalInput")
    transT_d = nc.dram_tensor("transT", [K, K], FP32, kind="ExternalInput")
    iota_neg_d = nc.dram_tensor("iota_neg", [BLOC, K], FP32, kind="ExternalInput")
    tags_d = nc.dram_tensor("tags", [BLOC, OUT_T], INT32, kind="ExternalOutput")
    lat_d = nc.dram_tensor("lat", [t_steps, 128, 64], FP32)

    with tile.TileContext(nc) as tc:
        with ExitStack() as ctx:
            static_pool = ctx.enter_context(tc.tile_pool(name="static", bufs=1))
            state_pool = ctx.enter_context(tc.tile_pool(name="state", bufs=3))
            acc_pool = ctx.enter_context(tc.tile_pool(name="acc", bufs=2))
            m_pool = ctx.enter_context(tc.tile_pool(name="m", bufs=2))
            g_pool = ctx.enter_context(tc.tile_pool(name="g", bufs=2))
            h_pool = ctx.enter_context(tc.tile_pool(name="h", bufs=2))
            if CH_SPACE == "PSUM":
                ch_pool = ctx.enter_context(tc.psum_pool(name="ch", bufs=2))
            else:
                ch_pool = ctx.enter_context(tc.tile_pool(name="ch", bufs=2))
            em_pool = ctx.enter_context(tc.tile_pool(name="em", bufs=6))
            bt_pool = ctx.enter_context(tc.tile_pool(name="bt", bufs=12))
            sm_pool = ctx.enter_context(tc.tile_pool(name="sm", bufs=6))

            # ---- static loads ----
            trans_rep = static_pool.tile([128, 64, K], FP32)
            nc.sync.dma_start(trans_rep[:], trans_rep_d.ap().transpose([1, 0, 2]))
            iota_neg = static_pool.tile([BLOC, K], FP32)
            nc.sync.dma_start(iota_neg[:], iota_neg_d.ap())
            CHB = [(BLOC * c // NCHAIN, BLOC * (c + 1) // NCHAIN) for c in range(NCHAIN)]
            tags_fc = [
                static_pool.tile([hi - lo, T], FP32, name=f"tagsf{c}", tag=f"tagsf{c}")
                for c, (lo, hi) in enumerate(CHB)
            ]

            em_tiles = {}

            def em_load(t):
                if t >= t_steps:
                    return
                em_t = em_pool.tile([128, 64], FP32)
                nc.sync.dma_start(em_t[:], em_f_d.ap()[t])
                em_tiles[t] = em_t

            # ---- t = 0 ----
            s = state_pool.tile([128, 64], FP32)
            nc.sync.dma_start(s[:], em_f_d.ap()[0])
            nc.sync.dma_start(lat_d.ap()[0], em_f_d.ap()[0])
            for t in (1, 2, 3):
                em_load(t)

            # ---- forward scan ----
            for t in range(1, t_steps):
                # DVE stt head
                acc = acc_pool.tile([128, K], FP32)
                nc.vector.tensor_scalar(
                    acc[:], trans_rep[:, 0, :], s[:, 0:1], None, op0=A.add
                )
                em_load(t + 3)
                for k in range(1, N_DVE):
                    nc.vector.scalar_tensor_tensor(
                        acc[:], trans_rep[:, k, :], s[:, k : k + 1], acc[:],
                        op0=A.add, op1=A.max,
                    )
                # Act producer planes (all issued up-front on Act queue)
                k0 = N_DVE
                chs = []
                for nk in chunk_sizes:
                    ch = ch_pool.tile([128, K, nk], FP32)
                    for kk in range(nk):
                        nc.scalar.add(
                            ch[:, :, kk],
                            trans_rep[:, k0 + kk, :],
                            s[:, k0 + kk : k0 + kk + 1],
                        )
                    chs.append(ch)
                    k0 += nk
                # DVE chunk reduces, merged into running max (starts from stt acc)
                macc = acc
                for ci, ch in enumerate(chs):
                    mc = m_pool.tile([128, K], FP32, name=f"mc{ci}", tag="mc")
                    nc.vector.tensor_reduce(
                        mc[:], ch[:], axis=mybir.AxisListType.X, op=A.max)
                    m2 = m_pool.tile([128, K], FP32, name=f"mm{ci}", tag="mm")
                    nc.vector.tensor_tensor(out=m2[:], in0=macc[:], in1=mc[:], op=A.max)
                    macc = m2
                # 4-group combine: g1 via DVE out-offset ts (no sem wait),
                # g2/g3 via Act copies (hidden under DVE TTs)
                g1 = g_pool.tile([32, K], FP32, name="g1", tag="g1")
                g2 = g_pool.tile([32, K], FP32, name="g2", tag="g2")
                g3 = g_pool.tile([32, K], FP32, name="g3", tag="g3")
                nc.vector.tensor_scalar(g1[:], macc[32:64, :], 0.0, None, op0=A.add)
                nc.scalar.copy(g2[:], macc[64:96, :])
                nc.scalar.copy(g3[:], macc[96:128, :])
                c1 = g_pool.tile([32, K], FP32, name="c1", tag="c1")
                nc.vector.tensor_tensor(out=c1[:], in0=macc[0:32, :], in1=g1[:], op=A.max)
                nc.vector.tensor_tensor(out=c1[:], in0=c1[:], in1=g2[:], op=A.max)
                m32 = g_pool.tile([32, K], FP32, name="m32", tag="m32")
                nc.vector.tensor_tensor(out=m32[:], in0=c1[:], in1=g3[:], op=A.max)
                # fold: DVE out-offset ts shift + aligned TT add (no cross-engine)
                em_t = em_tiles.pop(t)
                h = h_pool.tile([128, 64], FP32)
                s = state_pool.tile([128, 64], FP32)
                for ic in range(4):
                    nc.vector.tensor_scalar(
                        h[ic * 32 : (ic + 1) * 32, :],
                        m32[:, ic * 64 : (ic + 1) * 64], 0.0, None, op0=A.add,
                    )
                    nc.vector.tensor_tensor(
                        out=s[ic * 32 : (ic + 1) * 32, :],
                        in0=h[ic * 32 : (ic + 1) * 32, :],
                        in1=em_t[ic * 32 : (ic + 1) * 32, :],
                        op=A.add,
                    )
                nc.sync.dma_start(lat_d.ap()[t], s[:])

            # ---- backtrack ----
            def lat_rows(t, lo, hi):
                return lat_d.ap()[t].rearrange("(ic bb) k -> bb ic k", ic=4)[lo:hi]

            def argmax_step(val, t_col, c):
                nb = CHB[c][1] - CHB[c][0]
                m = sm_pool.tile([nb, 1], FP32, name=f"am{c}", tag=f"m{c}")
                nc.vector.tensor_reduce(m[:], val[:], axis=mybir.AxisListType.X, op=A.max)
                d = sm_pool.tile([nb, K], FP32, name=f"ad{c}", tag=f"d{c}")
                nc.vector.scalar_tensor_tensor(
                    d[:], val[:], m[:], iota_neg[0:nb, :], op0=A.is_ge, op1=A.mult
                )
                dmin = sm_pool.tile([nb, 1], FP32, name=f"admin{c}", tag=f"dmin{c}")
                nc.vector.tensor_reduce(dmin[:], d[:], axis=mybir.AxisListType.X, op=A.min)
                nc.scalar.copy(tags_fc[c][:, t_col : t_col + 1], dmin[:])
                idx = sm_pool.tile([nb, 1], INT32, name=f"aidx{c}", tag=f"idx{c}")
                nc.vector.tensor_scalar(idx[:], dmin[:], BIGN, None, op0=A.add)
                return idx

            idxs = [None] * NCHAIN
            for c, (lo, hi) in enumerate(CHB):
                sv = bt_pool.tile([hi - lo, K], FP32, name=f"sv{c}", tag=f"sv{c}")
                nc.sync.dma_start(sv[:], lat_rows(t_steps - 1, lo, hi))
                idxs[c] = argmax_step(sv, t_steps - 1, c)

            for t in range(t_steps - 2, -1, -1):
                svs = []
                for c, (lo, hi) in enumerate(CHB):
                    sv = bt_pool.tile([hi - lo, K], FP32, name=f"svl{c}", tag=f"sv{c}")
                    eng = nc.sync if c % 2 == 0 else nc.scalar
                    eng.dma_start(sv[:], lat_rows(t, lo, hi))
                    nc.gpsimd.indirect_dma_start(
                        out=sv[:],
                        out_offset=None,
                        in_=transT_d.ap(),
                        in_offset=bass.IndirectOffsetOnAxis(ap=idxs[c][:, :1], axis=0),
                        compute_op=A.add,
                    )
                    svs.append(sv)
                for c in range(NCHAIN):
                    idxs[c] = argmax_step(svs[c], t, c)

            # ---- output assembly (per chain) ----
            for c, (lo, hi) in enumerate(CHB):
                tags_i = static_pool.tile(
                    [hi - lo, OUT_T], INT32, name=f"tagsi{c}", tag=f"tagsi{c}"
                )
                nc.vector.memset(tags_i[:], 0)
                nc.vector.tensor_scalar(
                    tags_i[:, 0:t_steps], tags_fc[c][:, 0:t_steps], BIGN, None, op0=A.add
                )
                nc.sync.dma_start(tags_d.ap()[lo:hi, :], tags_i[:])

    nc.compile()
    return nc


def _prep_inputs(emissions, transitions, t_steps: int = T):
    emissions = np.ascontiguousarray(emissions[:, :t_steps, :], dtype=np.float32)
    transitions = np.ascontiguousarray(transitions, dtype=np.float32)

    tr = transitions.reshape(4, 64, K).transpose(1, 0, 2)
    trans_rep = np.broadcast_to(tr[:, :, None, :], (64, 4, BLOC, K)).reshape(64, 128, K)
    trans_rep = np.ascontiguousarray(trans_rep)
    transT = np.ascontiguousarray(transitions.T)
    iota_neg = np.ascontiguousarray(
        np.broadcast_to((np.arange(K, dtype=np.float32) - BIGN)[None, :], (BLOC, K))
    )

    in_maps = []
    for c in range(NCORES):
        em_c = emissions[c * BLOC : (c + 1) * BLOC]
        em_f = np.ascontiguousarray(
            em_c.reshape(BLOC, t_steps, 4, 64)
            .transpose(1, 2, 0, 3)
            .reshape(t_steps, 128, 64)
        )
        in_maps.append(
            {"em_f": em_f, "trans_rep": trans_rep, "transT": transT,
             "iota_neg": iota_neg}
        )
    return in_maps


def kernel(emissions, transitions, mask, max_sequence_length):
    from concourse.bass_utils import run_bass_kernel_spmd

    emissions = np.asarray(emissions)
    transitions = np.asarray(transitions)
    mask = np.asarray(mask)

    nc = build_program(T)
    in_maps = _prep_inputs(emissions, transitions, T)
    res = run_bass_kernel_spmd(nc, in_maps, list(range(NCORES)))
    tags = np.concatenate([res.results[c]["tags"] for c in range(NCORES)], axis=0)
    tags = tags.astype(np.int32)
    tags[:, :T] *= mask.astype(np.int32)
    return tags


# revision 6
# speedup vs baseline: 1.0539x; 1.0539x over previous
"""CRF Viterbi decode — v2: Act-producer + DVE stt/reduce hybrid forward.

Forward per step (partitions p=(ic*32+b); group ic reduces i in [64ic,64ic+64)):
  - DVE stt chain for k in [0, N_DVE) -> acc [128,256]
  - Act produces planes k in [N_DVE, 64): ch[:, :, kk] = trans_rep[:,k,:] + s[:,k]
    (strided writes into PSUM chunk tiles [128, 256, 8] at 477ns/plane)
  - DVE tensor_reduce max per chunk (contiguous-k inner axis, 1.08ns/elem),
    merged into a running max via TT (starts from the stt acc)
  - 4-group combine: Act shift-copies groups 1..3 to partition-base-0 tiles
    (inputs of engine ops must share a partition base; outputs may be offset),
    then 3 aligned TT maxes -> m32 [32,256]
  - fold: Act copies m32 col-blocks to partition offsets, DVE adds em -> s_next
  - lat store via DMA (off critical path)
Backtrack: recompute-argmax chains (no stored backpointers); SWDGE transT-row
gather fused with +s_t (compute_op=add); argmax via max8/max_index (first-index
tie semantics verified on HW); idx bitcast uint32->int32 feeds the next gather.
"""

import numpy as np

B, T, K = 256, 512, 256
NCORES = 8
BLOC = B // NCORES  # 32
OUT_T = T + 2  # 514
NCHAIN = 2

N_DVE = 16          # k's on DVE stt chain
CHUNK = 8           # planes per Act chunk (PSUM tile = CHUNK KB/partition)
CH_SPACE = "PSUM"   # chunk tile space: Act strided->PSUM measured 477ns/plane


def build_program(t_steps: int = T):
    from contextlib import ExitStack

    import concourse.bass as bass
    import concourse.tile as tile
    from concourse import bacc, mybir

    FP32 = mybir.dt.float32
    INT32 = mybir.dt.int32
    UINT32 = mybir.dt.uint32
    A = mybir.AluOpType

    n_act = 64 - N_DVE
    chunk_sizes = []
    rem = n_act
    while rem > 0:
        chunk_sizes.append(min(CHUNK, rem))
        rem -= min(CHUNK, rem)

    nc = bacc.Bacc("TRN2", target_bir_lowering=False, num_devices=NCORES)

    em_f_d = nc.dram_tensor("em_f", [t_steps, 128, 64], FP32, kind="ExternalInput")
    trans_rep_d = nc.dram_tensor("trans_rep", [64, 128, K], FP32, kind="ExternalInput")
    transT_d = nc.dram_tensor("transT", [K, K], FP32, kind="ExternalInput")
    tags_d = nc.dram_tensor("tags", [BLOC, OUT_T], INT32, kind="ExternalOutput")
    lat_d = nc.dram_tensor("lat", [t_steps, 128, 64], FP32)

    with tile.TileContext(nc) as tc:
        with ExitStack() as ctx:
            static_pool = ctx.enter_context(tc.tile_pool(name="static", bufs=1))
            state_pool = ctx.enter_context(tc.tile_pool(name="state", bufs=3))
            acc_pool = ctx.enter_context(tc.tile_pool(name="acc", bufs=2))
            m_pool = ctx.enter_context(tc.tile_pool(name="m", bufs=2))
            g_pool = ctx.enter_context(tc.tile_pool(name="g", bufs=2))
            h_pool = ctx.enter_context(tc.tile_pool(name="h", bufs=2))
            if CH_SPACE == "PSUM":
                ch_pool = ctx.enter_context(tc.psum_pool(name="ch", bufs=2))
            else:
                ch_pool = ctx.enter_context(tc.tile_pool(name="ch", bufs=2))
            em_pool = ctx.enter_context(tc.tile_pool(name="em", bufs=6))
            bt_pool = ctx.enter_context(tc.tile_pool(name="bt", bufs=12))
            sm_pool = ctx.enter_context(tc.tile_pool(name="sm", bufs=6))

            # ---- static loads ----
            trans_rep = static_pool.tile([128, 64, K], FP32)
            nc.sync.dma_start(trans_rep[:], trans_rep_d.ap().transpose([1, 0, 2]))
            CHB = [(BLOC * c // NCHAIN, BLOC * (c + 1) // NCHAIN) for c in range(NCHAIN)]
            tags_u = [
                static_pool.tile([hi - lo, OUT_T], UINT32, name=f"tagsu{c}", tag=f"tagsu{c}")
                for c, (lo, hi) in enumerate(CHB)
            ]
            for tu in tags_u:
                nc.vector.memset(tu[:], 0)

            em_tiles = {}

            def em_load(t):
                if t >= t_steps:
                    return
                em_t = em_pool.tile([128, 64], FP32)
                nc.sync.dma_start(em_t[:], em_f_d.ap()[t])
                em_tiles[t] = em_t

            # ---- t = 0 ----
            s = state_pool.tile([128, 64], FP32)
            nc.sync.dma_start(s[:], em_f_d.ap()[0])
            nc.sync.dma_start(lat_d.ap()[0], em_f_d.ap()[0])
            for t in (1, 2, 3):
                em_load(t)

            # ---- forward scan ----
            STT_GROUPS = [6, 5, 5]  # stt issue groups interleaved with reds
            assert sum(STT_GROUPS) == N_DVE
            for t in range(1, t_steps):
                # Act producer planes (issued up-front on the Act queue so the
                # engine starts filling PSUM chunks at fold-done)
                k0 = N_DVE
                chs = []
                for nk in chunk_sizes:
                    ch = ch_pool.tile([128, K, nk], FP32)
                    for kk in range(nk):
                        nc.scalar.add(
                            ch[:, :, kk],
                            trans_rep[:, k0 + kk, :],
                            s[:, k0 + kk : k0 + kk + 1],
                        )
                    chs.append(ch)
                    k0 += nk
                # DVE: stt groups interleaved with early chunk reduces so the
                # PSUM slot WAR clears before Act needs to refill it; merge
                # chain runs over chunk partials, stt acc folded in last
                acc = acc_pool.tile([128, K], FP32)
                nc.vector.tensor_scalar(
                    acc[:], trans_rep[:, 0, :], s[:, 0:1], None, op0=A.add
                )
                em_load(t + 3)
                k = 1
                for kn in range(1, STT_GROUPS[0]):
                    nc.vector.scalar_tensor_tensor(
                        acc[:], trans_rep[:, k, :], s[:, k : k + 1], acc[:],
                        op0=A.add, op1=A.max)
                    k += 1
                mcs = []

                def red(ci):
                    mc = m_pool.tile([128, K], FP32, name=f"mc{ci}", tag="mc")
                    nc.vector.tensor_reduce(
                        mc[:], chs[ci][:], axis=mybir.AxisListType.X, op=A.max)
                    mcs.append(mc)

                red(0)
                for kn in range(STT_GROUPS[1]):
                    nc.vector.scalar_tensor_tensor(
                        acc[:], trans_rep[:, k, :], s[:, k : k + 1], acc[:],
                        op0=A.add, op1=A.max)
                    k += 1
                red(1)
                for kn in range(STT_GROUPS[2]):
                    nc.vector.scalar_tensor_tensor(
                        acc[:], trans_rep[:, k, :], s[:, k : k + 1], acc[:],
                        op0=A.add, op1=A.max)
                    k += 1
                macc = mcs[0]
                for ci in range(1, len(chs)):
                    m2 = m_pool.tile([128, K], FP32, name=f"mm{ci}", tag="mm")
                    nc.vector.tensor_tensor(
                        out=m2[:], in0=macc[:], in1=mcs[ci][:], op=A.max)
                    macc = m2
                    if ci + 1 < len(chs):
                        red(ci + 1)
                mfin = m_pool.tile([128, K], FP32, name="mfin", tag="mfin")
                nc.vector.tensor_tensor(out=mfin[:], in0=macc[:], in1=acc[:], op=A.max)
                # 2-level combine, all-DVE (out-offset ts shifts + aligned TT)
                gA = g_pool.tile([64, K], FP32, name="gA", tag="gA")
                nc.vector.tensor_scalar(gA[:], mfin[64:128, :], 0.0, None, op0=A.add)
                c2 = g_pool.tile([64, K], FP32, name="c2", tag="c2")
                nc.vector.tensor_tensor(out=c2[:], in0=mfin[0:64, :], in1=gA[:], op=A.max)
                gB = g_pool.tile([32, K], FP32, name="gB", tag="gB")
                nc.vector.tensor_scalar(gB[:], c2[32:64, :], 0.0, None, op0=A.add)
                m32 = g_pool.tile([32, K], FP32, name="m32", tag="m32")
                nc.vector.tensor_tensor(out=m32[:], in0=c2[0:32, :], in1=gB[:], op=A.max)
                # fold: DVE out-offset ts shift + aligned TT add with em
                em_t = em_tiles.pop(t)
                h = h_pool.tile([128, 64], FP32)
                s = state_pool.tile([128, 64], FP32)
                for ic in range(4):
                    nc.vector.tensor_scalar(
                        h[ic * 32 : (ic + 1) * 32, :],
                        m32[:, ic * 64 : (ic + 1) * 64], 0.0, None, op0=A.add,
                    )
                    nc.vector.tensor_tensor(
                        out=s[ic * 32 : (ic + 1) * 32, :],
                        in0=h[ic * 32 : (ic + 1) * 32, :],
                        in1=em_t[ic * 32 : (ic + 1) * 32, :],
                        op=A.add,
                    )
                nc.sync.dma_start(lat_d.ap()[t], s[:])

            # ---- backtrack ----
            def lat_rows(t, lo, hi):
                return lat_d.ap()[t].rearrange("(ic bb) k -> bb ic k", ic=4)[lo:hi]

            def argmax_step(val, t_col, c):
                nb = CHB[c][1] - CHB[c][0]
                m8 = sm_pool.tile([nb, 8], FP32, name=f"m8{c}", tag=f"m8{c}")
                nc.vector.max(m8[:], val[:])
                i8 = sm_pool.tile([nb, 8], UINT32, name=f"i8{c}", tag=f"i8{c}")
                nc.vector.max_index(i8[:], m8[:], val[:])
                nc.vector.tensor_copy(tags_u[c][:, t_col : t_col + 1], i8[:, 0:1])
                return i8

            idxs = [None] * NCHAIN
            for c, (lo, hi) in enumerate(CHB):
                sv = bt_pool.tile([hi - lo, K], FP32, name=f"sv{c}", tag=f"sv{c}")
                nc.sync.dma_start(sv[:], lat_rows(t_steps - 1, lo, hi))
                idxs[c] = argmax_step(sv, t_steps - 1, c)

            for t in range(t_steps - 2, -1, -1):
                svs = []
                for c, (lo, hi) in enumerate(CHB):
                    sv = bt_pool.tile([hi - lo, K], FP32, name=f"svl{c}", tag=f"sv{c}")
                    eng = nc.sync if c % 2 == 0 else nc.scalar
                    eng.dma_start(sv[:], lat_rows(t, lo, hi))
                    nc.gpsimd.indirect_dma_start(
                        out=sv[:],
                        out_offset=None,
                        in_=transT_d.ap(),
                        in_offset=bass.IndirectOffsetOnAxis(
                            ap=idxs[c][:, 0:1].bitcast(INT32), axis=0
                        ),
                        compute_op=A.add,
                    )
                    svs.append(sv)
                for c in range(NCHAIN):
                    idxs[c] = argmax_step(svs[c], t, c)

            # ---- output ----
            for c, (lo, hi) in enumerate(CHB):
                nc.sync.dma_start(tags_d.ap()[lo:hi, :], tags_u[c][:].bitcast(INT32))

    nc.compile()
    return nc


def _prep_inputs(emissions, transitions, t_steps: int = T):
    emissions = np.ascontiguousarray(emissions[:, :t_steps, :], dtype=np.float32)
    transitions = np.ascontiguousarray(transitions, dtype=np.float32)

    tr = transitions.reshape(4, 64, K).transpose(1, 0, 2)
    trans_rep = np.broadcast_to(tr[:, :, None, :], (64, 4, BLOC, K)).reshape(64, 128, K)
    trans_rep = np.ascontiguousarray(trans_rep)
    transT = np.ascontiguousarray(transitions.T)

    in_maps = []
    for c in range(NCORES):
        em_c = emissions[c * BLOC : (c + 1) * BLOC]
        em_f = np.ascontiguousarray(
            em_c.reshape(BLOC, t_steps, 4, 64)
            .transpose(1, 2, 0, 3)
            .reshape(t_steps, 128, 64)
        )
        in_maps.append({"em_f": em_f, "trans_rep": trans_rep, "transT": transT})
    return in_maps


def kernel(emissions, transitions, mask, max_sequence_length):
    from concourse.bass_utils import run_bass_kernel_spmd

    emissions = np.asarray(emissions)
    transitions = np.asarray(transitions)
    mask = np.asarray(mask)

    nc = build_program(T)
    in_maps = _prep_inputs(emissions, transitions, T)
    res = run_bass_kernel_spmd(nc, in_maps, list(range(NCORES)))
    tags = np.concatenate([res.results[c]["tags"] for c in range(NCORES)], axis=0)
    tags = tags.astype(np.int32)
    tags[:, :T] *= mask.astype(np.int32)
    return tags


# revision 8
# speedup vs baseline: 1.0659x; 1.0114x over previous
"""CRF Viterbi decode — v2: Act-producer + DVE stt/reduce hybrid forward.

Forward per step (partitions p=(ic*32+b); group ic reduces i in [64ic,64ic+64)):
  - DVE stt chain for k in [0, N_DVE) -> acc [128,256]
  - Act produces planes k in [N_DVE, 64): ch[:, :, kk] = trans_rep[:,k,:] + s[:,k]
    (strided writes into PSUM chunk tiles [128, 256, 8] at 477ns/plane)
  - DVE tensor_reduce max per chunk (contiguous-k inner axis, 1.08ns/elem),
    merged into a running max via TT (starts from the stt acc)
  - 4-group combine: Act shift-copies groups 1..3 to partition-base-0 tiles
    (inputs of engine ops must share a partition base; outputs may be offset),
    then 3 aligned TT maxes -> m32 [32,256]
  - fold: Act copies m32 col-blocks to partition offsets, DVE adds em -> s_next
  - lat store via DMA (off critical path)
Backtrack: recompute-argmax chains (no stored backpointers); SWDGE transT-row
gather fused with +s_t (compute_op=add); argmax via max8/max_index (first-index
tie semantics verified on HW); idx bitcast uint32->int32 feeds the next gather.
"""

import numpy as np

B, T, K = 256, 512, 256
NCORES = 8
BLOC = B // NCORES  # 32
OUT_T = T + 2  # 514
NCHAIN = 2

N_DVE = 16          # k's on DVE stt chain
CHUNK = 8           # planes per Act chunk (PSUM tile = CHUNK KB/partition)
CH_SPACE = "PSUM"   # chunk tile space: Act strided->PSUM measured 477ns/plane


def build_program(t_steps: int = T):
    from contextlib import ExitStack

    import concourse.bass as bass
    import concourse.tile as tile
    from concourse import bacc, mybir

    FP32 = mybir.dt.float32
    INT32 = mybir.dt.int32
    UINT32 = mybir.dt.uint32
    A = mybir.AluOpType

    n_act = 64 - N_DVE
    chunk_sizes = []
    rem = n_act
    while rem > 0:
        chunk_sizes.append(min(CHUNK, rem))
        rem -= min(CHUNK, rem)

    nc = bacc.Bacc("TRN2", target_bir_lowering=False, num_devices=NCORES)

    em_f_d = nc.dram_tensor("em_f", [t_steps, 128, 64], FP32, kind="ExternalInput")
    trans_rep_d = nc.dram_tensor("trans_rep", [64, 128, K], FP32, kind="ExternalInput")
    transT_d = nc.dram_tensor("transT", [K, K], FP32, kind="ExternalInput")
    tags_d = nc.dram_tensor("tags", [BLOC, OUT_T], INT32, kind="ExternalOutput")
    lat_d = nc.dram_tensor("lat", [t_steps, 128, 64], FP32)

    with tile.TileContext(nc) as tc:
        with ExitStack() as ctx:
            static_pool = ctx.enter_context(tc.tile_pool(name="static", bufs=1))
            state_pool = ctx.enter_context(tc.tile_pool(name="state", bufs=3))
            acc_pool = ctx.enter_context(tc.tile_pool(name="acc", bufs=2))
            m_pool = ctx.enter_context(tc.tile_pool(name="m", bufs=2))
            g_pool = ctx.enter_context(tc.tile_pool(name="g", bufs=2))
            h_pool = ctx.enter_context(tc.tile_pool(name="h", bufs=2))
            if CH_SPACE == "PSUM":
                ch_pool = ctx.enter_context(tc.psum_pool(name="ch", bufs=2))
            else:
                ch_pool = ctx.enter_context(tc.tile_pool(name="ch", bufs=2))
            em_pool = ctx.enter_context(tc.tile_pool(name="em", bufs=6))
            bt_pool = ctx.enter_context(tc.tile_pool(name="bt", bufs=12))
            sm_pool = ctx.enter_context(tc.tile_pool(name="sm", bufs=6))

            # ---- static loads ----
            trans_rep = static_pool.tile([128, 64, K], FP32)
            nc.sync.dma_start(trans_rep[:], trans_rep_d.ap().transpose([1, 0, 2]))
            CHB = [(BLOC * c // NCHAIN, BLOC * (c + 1) // NCHAIN) for c in range(NCHAIN)]
            tags_u = [
                static_pool.tile([hi - lo, OUT_T], UINT32, name=f"tagsu{c}", tag=f"tagsu{c}")
                for c, (lo, hi) in enumerate(CHB)
            ]
            for tu in tags_u:
                nc.vector.memset(tu[:], 0)

            em_tiles = {}

            def em_load(t):
                if t >= t_steps:
                    return
                em_t = em_pool.tile([128, 64], FP32)
                nc.sync.dma_start(em_t[:], em_f_d.ap()[t])
                em_tiles[t] = em_t

            # ---- t = 0 ----
            s = state_pool.tile([128, 64], FP32)
            nc.sync.dma_start(s[:], em_f_d.ap()[0])
            nc.sync.dma_start(lat_d.ap()[0], em_f_d.ap()[0])
            for t in (1, 2, 3):
                em_load(t)

            # ---- forward scan ----
            STT_GROUPS = [6, 5, 5]  # stt issue groups interleaved with reds
            assert sum(STT_GROUPS) == N_DVE
            for t in range(1, t_steps):
                # Act producer planes (issued up-front on the Act queue so the
                # engine starts filling PSUM chunks at fold-done)
                k0 = N_DVE
                chs = []
                for nk in chunk_sizes:
                    ch = ch_pool.tile([128, K, nk], FP32)
                    for kk in range(nk):
                        nc.scalar.add(
                            ch[:, :, kk],
                            trans_rep[:, k0 + kk, :],
                            s[:, k0 + kk : k0 + kk + 1],
                        )
                    chs.append(ch)
                    k0 += nk
                # DVE: stt groups interleaved with early chunk reduces so the
                # PSUM slot WAR clears before Act needs to refill it; merge
                # chain runs over chunk partials, stt acc folded in last
                acc = acc_pool.tile([128, K], FP32)
                nc.vector.tensor_scalar(
                    acc[:], trans_rep[:, 0, :], s[:, 0:1], None, op0=A.add
                )
                em_load(t + 3)
                k = 1
                for kn in range(1, STT_GROUPS[0]):
                    nc.vector.scalar_tensor_tensor(
                        acc[:], trans_rep[:, k, :], s[:, k : k + 1], acc[:],
                        op0=A.add, op1=A.max)
                    k += 1
                mcs = []

                def red(ci):
                    mc = m_pool.tile([128, K], FP32, name=f"mc{ci}", tag="mc")
                    nc.vector.tensor_reduce(
                        mc[:], chs[ci][:], axis=mybir.AxisListType.X, op=A.max)
                    mcs.append(mc)

                red(0)
                for kn in range(STT_GROUPS[1]):
                    nc.vector.scalar_tensor_tensor(
                        acc[:], trans_rep[:, k, :], s[:, k : k + 1], acc[:],
                        op0=A.add, op1=A.max)
                    k += 1
                red(1)
                for kn in range(STT_GROUPS[2]):
                    nc.vector.scalar_tensor_tensor(
                        acc[:], trans_rep[:, k, :], s[:, k : k + 1], acc[:],
                        op0=A.add, op1=A.max)
                    k += 1
                # merge chain: fold the stt acc in EARLY so only one TT remains
                # serial after the last chunk's reduce
                macc = m_pool.tile([128, K], FP32, name="mm0", tag="mm")
                nc.vector.tensor_tensor(
                    out=macc[:], in0=mcs[0][:], in1=mcs[1][:], op=A.max)
                m2 = m_pool.tile([128, K], FP32, name="mma", tag="mm")
                nc.vector.tensor_tensor(out=m2[:], in0=macc[:], in1=acc[:], op=A.max)
                macc = m2
                for ci in range(2, len(chs)):
                    red(ci)
                    m2 = m_pool.tile([128, K], FP32, name=f"mm{ci}", tag="mm")
                    nc.vector.tensor_tensor(
                        out=m2[:], in0=macc[:], in1=mcs[ci][:], op=A.max)
                    macc = m2
                mfin = macc
                # 2-level combine, all-DVE (out-offset ts shifts + aligned TT)
                gA = g_pool.tile([64, K], FP32, name="gA", tag="gA")
                nc.vector.tensor_scalar(gA[:], mfin[64:128, :], 0.0, None, op0=A.add)
                c2 = g_pool.tile([64, K], FP32, name="c2", tag="c2")
                nc.vector.tensor_tensor(out=c2[:], in0=mfin[0:64, :], in1=gA[:], op=A.max)
                gB = g_pool.tile([32, K], FP32, name="gB", tag="gB")
                nc.vector.tensor_scalar(gB[:], c2[32:64, :], 0.0, None, op0=A.add)
                m32 = g_pool.tile([32, K], FP32, name="m32", tag="m32")
                nc.vector.tensor_tensor(out=m32[:], in0=c2[0:32, :], in1=gB[:], op=A.max)
                # fold by column halves: s[:, 0:32] ready first so the next
                # step's stt head and first Act chunks launch during half 2
                em_t = em_tiles.pop(t)
                h = h_pool.tile([128, 64], FP32)
                s = state_pool.tile([128, 64], FP32)
                for half in (0, 1):
                    kl, kh = half * 32, half * 32 + 32
                    for ic in range(4):
                        nc.vector.tensor_scalar(
                            h[ic * 32 : (ic + 1) * 32, kl:kh],
                            m32[:, ic * 64 + kl : ic * 64 + kh], 0.0, None, op0=A.add,
                        )
                    nc.vector.tensor_tensor(
                        out=s[:, kl:kh], in0=h[:, kl:kh], in1=em_t[:, kl:kh], op=A.add
                    )
                nc.sync.dma_start(lat_d.ap()[t], s[:])

            # ---- backtrack ----
            def lat_rows(t, lo, hi):
                return lat_d.ap()[t].rearrange("(ic bb) k -> bb ic k", ic=4)[lo:hi]

            def argmax_step(val, t_col, c):
                nb = CHB[c][1] - CHB[c][0]
                m8 = sm_pool.tile([nb, 8], FP32, name=f"m8{c}", tag=f"m8{c}")
                nc.vector.max(m8[:], val[:])
                i8 = sm_pool.tile([nb, 8], UINT32, name=f"i8{c}", tag=f"i8{c}")
                nc.vector.max_index(i8[:], m8[:], val[:])
                nc.vector.tensor_copy(tags_u[c][:, t_col : t_col + 1], i8[:, 0:1])
                return i8

            idxs = [None] * NCHAIN
            for c, (lo, hi) in enumerate(CHB):
                sv = bt_pool.tile([hi - lo, K], FP32, name=f"sv{c}", tag=f"sv{c}")
                nc.sync.dma_start(sv[:], lat_rows(t_steps - 1, lo, hi))
                idxs[c] = argmax_step(sv, t_steps - 1, c)

            for t in range(t_steps - 2, -1, -1):
                svs = []
                for c, (lo, hi) in enumerate(CHB):
                    sv = bt_pool.tile([hi - lo, K], FP32, name=f"svl{c}", tag=f"sv{c}")
                    eng = nc.sync if c % 2 == 0 else nc.scalar
                    eng.dma_start(sv[:], lat_rows(t, lo, hi))
                    nc.gpsimd.indirect_dma_start(
                        out=sv[:],
                        out_offset=None,
                        in_=transT_d.ap(),
                        in_offset=bass.IndirectOffsetOnAxis(
                            ap=idxs[c][:, 0:1].bitcast(INT32), axis=0
                        ),
                        compute_op=A.add,
                    )
                    svs.append(sv)
                for c in range(NCHAIN):
                    idxs[c] = argmax_step(svs[c], t, c)

            # ---- output ----
            for c, (lo, hi) in enumerate(CHB):
                nc.sync.dma_start(tags_d.ap()[lo:hi, :], tags_u[c][:].bitcast(INT32))

    nc.compile()
    return nc


def _prep_inputs(emissions, transitions, t_steps: int = T):
    emissions = np.ascontiguousarray(emissions[:, :t_steps, :], dtype=np.float32)
    transitions = np.ascontiguousarray(transitions, dtype=np.float32)

    tr = transitions.reshape(4, 64, K).transpose(1, 0, 2)
    trans_rep = np.broadcast_to(tr[:, :, None, :], (64, 4, BLOC, K)).reshape(64, 128, K)
    trans_rep = np.ascontiguousarray(trans_rep)
    transT = np.ascontiguousarray(transitions.T)

    in_maps = []
    for c in range(NCORES):
        em_c = emissions[c * BLOC : (c + 1) * BLOC]
        em_f = np.ascontiguousarray(
            em_c.reshape(BLOC, t_steps, 4, 64)
            .transpose(1, 2, 0, 3)
            .reshape(t_steps, 128, 64)
        )
        in_maps.append({"em_f": em_f, "trans_rep": trans_rep, "transT": transT})
    return in_maps


def kernel(emissions, transitions, mask, max_sequence_length):
    from concourse.bass_utils import run_bass_kernel_spmd

    emissions = np.asarray(emissions)
    transitions = np.asarray(transitions)
    mask = np.asarray(mask)

    nc = build_program(T)
    in_maps = _prep_inputs(emissions, transitions, T)
    res = run_bass_kernel_spmd(nc, in_maps, list(range(NCORES)))
    tags = np.concatenate([res.results[c]["tags"] for c in range(NCORES)], axis=0)
    tags = tags.astype(np.int32)
    tags[:, :T] *= mask.astype(np.int32)
    return tags
